# revision 65
# baseline (speedup 1.0000x reference)
"""SAGAN-style self-attention on 8 trn2 cores: data-parallel over batch.

Per core (one batch image): x^T [256,4096] bf16 in, out^T [256,4096] f32 out.
Projections (Q/K/V) chase the x DMAs (round-robin across SP/ACT/Pool
triggers); tile 0's attention units interleave with the tail of that chase.

  QT/KT = W^T @ xT + b        [32, 4096]  (bias via DVE tensor_scalar; K goes
                              straight into per-group kt_stack row strips)
  V     = x @ Wh + bh         fp8e4 pair tiles [128, 2, 256] ([keys, pair, c])
  per 512-query tile, per unit = pair of 128-key blocks (16 units/tile):
    T    = KT_strips.T @ QT    [128 keys, 2*512] 2-way row-packed (K=32),
                               t psum double-buffered so exp(u) || T(u+1)
    expT = exp(T)              ScalarE, PSUM->SBUF, fp8e4 (|s|<~5, no max-sub)
    O'  += V2_pair.T @ E2_pair [256, 512] fp8 DoubleRow PSUM accum
    Z   += ones2.T @ E2_pair   [16, 512] fp8 DoubleRow (rows identical)
  tail (deferred, off the PE critical path; O'/Z of the next tile are held
  back a few units so PSUM-evacuation WARs are covered by T-pack work):
    osb  = O' -> bf16 (DVE, frees o banks early); zf32 = Z -> SBUF (frees zz)
    zr   = 1/Z (DVE reciprocal, reads the SBUF copy)
    f    = bo (x) Z (K=1 preload) + Wo^T @ osb   (normalize AFTER projection:
    fraw = f -> bf16 (DVE, frees the f bank)      the division commutes)
    zb   = ones (x) (1/Z) broadcast [128, 512] (PE, borrows the f slot)
    out  = fraw * zb (DVE) -> DMA
PSUM banks: t=2x2 (double-buffered T), o0/o1=2, zz=1, f=1 -> 8 exactly.
fp8 E/V numerics: rel-L2 vs f64 reference = 1.47e-2 (gate 2e-2).

Tuning on top of the 215.9us baseline (measured 207.6us):
  - DMA triggers come off ScalarE once attention starts (exp owns ACT;
    each trigger costs ~600ns of the issuing engine's sequencer)
  - O' weights stored SW-interleaved ([keys, C, pair] with Wh/bh channel-
    reversed per 128-half on the host) and consumed with
    perf_mode=DoubleRowSwInterleave: weight loads read contiguously,
    LDWEIGHTS 213 -> ~90-130ns (O' issue rate itself is drain-bound, so
    the win is modest)
  - V pairs 4-7 are front-loaded into the prologue: their MMs bridge the
    PE-idle window while the kt_stack/qts strip DMAs land; without them
    the HAM MID window fires and the first ~10us of attention runs at
    K=4/8 (half clock)
  - rampdown tail (was 11.7us after the last O'): the final tile computes
    1/Z on ScalarE as exp(-ln Z) (Ln shares the exp table set, no
    ACT_TABLE_LOAD; the 3.3us single-lane DVE reciprocal was blocking the
    fr CASTs on the DVE queue), its out-multiply reads the zb broadcast
    psum directly, and its second out-proj half borrows the freed zz bank
    so both f halves' matmuls run without the serial pf-bank rotation
  - Z emitted BEFORE the unit's two O' matmuls (was after): Z's 32-col
    ones LDWEIGHTS pulls ahead during the T-pair streams and the Z matmul
    starts inside the T-pair's drain shadow, while the larger DRSI O'
    weight loads stream during Z's execution — hides most of Z's ~300ns
    (-8us). Emitting the next T-pack after the OZ group instead regresses
    13us (ScalarE starves at boundaries), so pack-ahead stays.
Measured 193.8-196.7us over three runs, rel-L2 identical to baseline.
"""

import sys

if "/opt/trn_rl_repo" not in sys.path:
    sys.path.insert(0, "/opt/trn_rl_repo")

import ml_dtypes
import numpy as np

import concourse.bass as bass
import concourse.mybir as mybir
import concourse.tile as tile
from concourse.bass_utils import run_bass_kernel_spmd

B, H, W, C = 8, 64, 64, 256
KEY = 32
N = H * W          # 4096 tokens
NT = 512           # query tile (free dim per matmul)
NTILES = N // NT   # 8
MB = 128           # key block
NMB = N // MB      # 32
GRP = 4            # key blocks per group (one per PE row strip)
NGRP = NMB // GRP  # 8

BF16 = mybir.dt.bfloat16
F32 = mybir.dt.float32
F8 = mybir.dt.float8e4
U8 = mybir.dt.uint8
FT = mybir.ActivationFunctionType
DR = mybir.MatmulPerfMode.DoubleRow
DRSI = mybir.MatmulPerfMode.DoubleRowSwInterleave

WD = 0             # exp fully on ScalarE (DVE insertion into the unit path regresses)
EXPA = 8.0 / float(np.log(2.0))
EXPB = 55.55       # zero-mean fp8e4m3 bit-trick bias (probe: exact u8 RN)


def build_nc() -> bass.Bass:
    nc = bass.Bass()

    xT = nc.declare_dram_parameter("xT", [2, 128, N], BF16, isOutput=False)
    wf = nc.declare_dram_parameter("wf", [2, 128, KEY], BF16, isOutput=False)
    wg = nc.declare_dram_parameter("wg", [2, 128, KEY], BF16, isOutput=False)
    wh = nc.declare_dram_parameter("wh", [2, 128, C], BF16, isOutput=False)
    wo = nc.declare_dram_parameter("wo", [2, 128, C], BF16, isOutput=False)
    bfT = nc.declare_dram_parameter("bfT", [KEY, 1], F32, isOutput=False)
    bgT = nc.declare_dram_parameter("bgT", [KEY, 1], F32, isOutput=False)
    bhp = nc.declare_dram_parameter("bhp", [1, C], BF16, isOutput=False)
    bop = nc.declare_dram_parameter("bop", [1, C], BF16, isOutput=False)
    outT = nc.declare_dram_parameter("outT", [2, 128, N], F32, isOutput=True)

    with tile.TileContext(nc) as tc:
        with (
            tc.tile_pool(name="const", bufs=1) as const,
            tc.tile_pool(name="xp", bufs=1) as xp,
            tc.tile_pool(name="vp", bufs=1) as vp,
            tc.tile_pool(name="qk", bufs=1) as qk,
            tc.tile_pool(name="ep", bufs=5) as ep,
            tc.tile_pool(name="osb", bufs=2) as osbp,
            tc.tile_pool(name="frp", bufs=2) as frp,
            tc.tile_pool(name="zsp", bufs=2) as zsp,
            tc.tile_pool(name="outp", bufs=3) as outp,
            tc.tile_pool(name="pt", bufs=2, space="PSUM") as pt,
            tc.tile_pool(name="po", bufs=1, space="PSUM") as po,
            tc.tile_pool(name="pz", bufs=1, space="PSUM") as pz,
            tc.tile_pool(name="pf", bufs=1, space="PSUM") as pf,
        ):
            # ---- constants ----
            ones2 = const.tile([128, 2, 16], F8)    # Z DoubleRow lhsT
            # (16-wide: DR weights need pair-step %16B == 0; rows identical)
            ones_m = const.tile([1, 128], BF16)     # K=1 broadcast lhsT
            nc.vector.memset(ones2, 1.0)
            nc.vector.memset(ones_m, 1.0)

            wf_sb = const.tile([128, 2, KEY], BF16)
            wg_sb = const.tile([128, 2, KEY], BF16)
            wh_sb = const.tile([128, 2, C], BF16)
            wo_sb = const.tile([128, 2, C], BF16)
            bf_sb = const.tile([KEY, 1], F32)
            bg_sb = const.tile([KEY, 1], F32)
            bh_sb = const.tile([1, C], BF16)
            bo_sb = const.tile([1, C], BF16)
            # DMA trigger round-robin across the three HWDGE/SWDGE engines:
            # each trigger costs ~600ns of that engine's sequencer, and a
            # single queue would serialize the whole input load (~40us).
            dma_i = 0
            dma_engs = [nc.sync, nc.scalar, nc.gpsimd]

            def dma_rr(out, in_):
                nonlocal dma_i
                eng = dma_engs[dma_i % len(dma_engs)]
                dma_i += 1
                eng.dma_start(out=out, in_=in_)
            # bh broadcast to all 128 partitions (for V bias add on DVE),
            # via K=1 ones matmul into a borrowed PSUM slot
            bh_bc = const.tile([128, C], BF16)

            # xT chunks in 512-col tiles (projections start on first slice)
            xts = [
                [xp.tile([128, NT], BF16, name=f"xt{cc}_{h}") for h in range(NTILES)]
                for cc in range(2)
            ]
            # x chunks first (K proj group g needs x tile h=g), weights
            # interleaved right behind the first x pair
            def xdma(h):
                for cc in range(2):
                    dma_rr(xts[cc][h], xT[cc, :, h * NT:(h + 1) * NT])

            xdma(0)
            for cc in range(2):
                dma_rr(wg_sb[:, cc, :], wg[cc])
                dma_rr(wf_sb[:, cc, :], wf[cc])
            dma_rr(bg_sb, bgT[:])
            dma_rr(bf_sb, bfT[:])
            xdma(1)
            for cc in range(2):
                dma_rr(wh_sb[:, cc, :], wh[cc])
            dma_rr(bh_sb, bhp[:])
            xdma(2)
            for cc in range(2):
                dma_rr(wo_sb[:, cc, :], wo[cc])
            dma_rr(bo_sb, bop[:])
            for h in range(3, NTILES):
                xdma(h)

            def xs(cc, start, width):
                h = start // NT
                assert (start + width - 1) // NT == h
                return xts[cc][h][:, start - h * NT: start - h * NT + width]

            pp_i = 0

            def proj_psum(shape):
                # borrow the "t"/"f" slots (alternating) before attention
                nonlocal pp_i
                pp_i += 1
                if pp_i % 2:
                    return pt.tile(shape, F32, tag="t", name=f"projps{pp_i}")
                return pf.tile(shape, F32, tag="f", name=f"projps{pp_i}")

            def emit_bh_bc():
                bh_ps = proj_psum([128, C])
                nc.tensor.matmul(bh_ps, ones_m, bh_sb, start=True, stop=True)
                nc.vector.tensor_copy(out=bh_bc, in_=bh_ps)

            # ---- K projection; per-group kt_stack tiles (so T of group g
            # only waits for group g), regrouped into row strips by
            # partition-shifting SBUF->SBUF DMAs
            kt_stack = [
                qk.tile([128, MB], BF16, name=f"kstk{g}") for g in range(NGRP)
            ]

            def kproj(g):
                ps = proj_psum([KEY, NT])
                for cc in range(2):
                    nc.tensor.matmul(
                        ps, wg_sb[:, cc, :], xs(cc, g * NT, NT),
                        start=(cc == 0), stop=(cc == 1),
                    )
                ktg = qk.tile([KEY, NT], BF16, name=f"kts{g}")
                nc.vector.tensor_scalar_add(ktg, ps, bg_sb)
                for i in range(4):
                    dma_rr(
                        kt_stack[g][32 * i:32 * (i + 1), :],
                        ktg[:, i * MB:(i + 1) * MB],
                    )

            # ---- per-query-tile Q tiles (replicated into 4 row strips) -----
            qts = [
                qk.tile([128, NT], BF16, name=f"qts{nt}") for nt in range(NTILES)
            ]

            def f_psum(shape):
                nonlocal pp_i
                pp_i += 1
                return pf.tile(shape, F32, tag="f", name=f"fps{pp_i}")

            def qproj(nt, psum_fn=None):
                ps = (psum_fn or proj_psum)([KEY, NT])
                for cc in range(2):
                    nc.tensor.matmul(
                        ps, wf_sb[:, cc, :], xs(cc, nt * NT, NT),
                        start=(cc == 0), stop=(cc == 1),
                    )
                nc.vector.tensor_scalar_add(qts[nt][0:KEY, :], ps, bf_sb)
                for i in range(1, 4):
                    nc.sync.dma_start(
                        out=qts[nt][32 * i:32 * (i + 1), :], in_=qts[nt][0:KEY, :]
                    )

            # ---- V projection -> fp8 pair tiles [128, 2, C] ----------------
            # V pair tiles in SW-interleaved layout [keys, C, pair]: the two
            # key blocks' values for a channel sit in adjacent bytes, so the
            # O' weight load reads contiguously (DoubleRowSwInterleave) and
            # skips DoubleRow's slow 2x interleaved LDWEIGHTS. The mode reads
            # weight columns reversed; host prep stores Wh/bh channel-reversed
            # within each 128-half so O' output partitions come out straight.
            v2 = [
                vp.tile([128, C, 2], F8, tag=f"v{p}", name=f"v{p}")
                for p in range(NMB // 2)
            ]

            def v2_lhsT(u, cc):
                # contiguous [128, 2, 128] descriptor over the interleaved
                # 256B half (pair stride 128B, col stride 1B)
                return (
                    v2[u][:, cc * 128:(cc + 1) * 128, :]
                    .rearrange("p m j -> p (m j)")
                    .rearrange("p (a b) -> p a b", a=2)
                )

            def vpair(pair, psum_fn):
                for mem in range(2):
                    mb = 2 * pair + mem
                    ps = psum_fn()
                    for cc in range(2):
                        nc.tensor.matmul(
                            ps, xs(cc, mb * MB, MB), wh_sb[:, cc, :],
                            start=(cc == 0), stop=(cc == 1),
                        )
                    nc.vector.tensor_tensor(
                        out=v2[pair][:, :, mem], in0=ps, in1=bh_bc,
                        op=mybir.AluOpType.add,
                    )

            # prologue: Q for tiles 0/1 and the first two K groups / four V
            # pairs; K groups 2..7 (+ V pairs) interleave with tile 0's units
            emit_bh_bc()
            qproj(0)
            qproj(1)
            for g in range(2):
                kproj(g)
                vpair(2 * g, lambda: proj_psum([128, C]))
                vpair(2 * g + 1, lambda: proj_psum([128, C]))
            # front-load V pairs 4-7: their x chunks are resident by now, so
            # these MMs bridge the PE-idle window while the kt_stack/qts strip
            # DMAs land. Without them PE idles ~2.3us waiting for the first T
            # pack, the HAM MID window fires, and the first ~10us of attention
            # runs at K=4/8 (half clock).
            for p4 in range(4, 8):
                vpair(p4, lambda: proj_psum([128, C]))

            # ---- attention: pipelined over (query-tile, pair-unit) ----------
            # unit u = one pair of key blocks (2u, 2u+1); 16 units per tile.
            # T psum is [128, 2*NT] (2 banks) double-buffered so exp(u) and
            # T(u+1) overlap; row strips alternate (0,1)/(2,3) across units.
            NU = NMB // 2  # 16
            state = {}  # nt -> dict with live tiles for the tail

            def emit_oz(nt, u, e_sb):
                if u == 0:
                    state[nt] = {
                        "o": [po.tile([128, NT], F32, tag="o0", name=f"o0_{nt}"),
                              po.tile([128, NT], F32, tag="o1", name=f"o1_{nt}")],
                        "zz": pz.tile([16, NT], F32, tag="z", name=f"z{nt}"),
                    }
                st = state[nt]
                first, last = u == 0, u == NU - 1
                nc.tensor.matmul(
                    st["zz"], ones2, e_sb,
                    start=first, stop=last,
                    perf_mode=DR,
                )
                for cc in range(2):
                    nc.tensor.matmul(
                        st["o"][cc],
                        v2_lhsT(u, cc),
                        e_sb,
                        start=first, stop=last,
                        perf_mode=DRSI,
                    )

            def tail1(nt):
                """PSUM evacuation; emit BEFORE next tile's first O'/Z.
                zz is freed by one fast copy (the slow reciprocal reads
                the SBUF copy later)."""
                st = state[nt]
                ot0 = osbp.tile([128, NT], BF16, tag="os0", name=f"os0_{nt}")
                nc.vector.tensor_copy(out=ot0, in_=st["o"][0])
                ot1 = osbp.tile([128, NT], BF16, tag="os1", name=f"os1_{nt}")
                nc.vector.tensor_copy(out=ot1, in_=st["o"][1])
                st["osb0"], st["osb1"] = ot0, ot1
                zf32 = zsp.tile([1, NT], F32, tag="zf32", name=f"zf32_{nt}")
                nc.vector.tensor_copy(out=zf32, in_=st["zz"][0:1, :])
                zbf = zsp.tile([1, NT], BF16, tag="zbf", name=f"zbf{nt}")
                nc.vector.tensor_copy(out=zbf, in_=zf32)
                zrb = zsp.tile([1, NT], BF16, tag="zrb", name=f"zrb{nt}")
                if nt == NTILES - 1:
                    # rampdown: the 3.3us DVE reciprocal would block the fr
                    # CASTs on the DVE queue; ScalarE is idle after the last
                    # exp, so compute 1/Z = exp(-ln Z) there instead
                    zl = zsp.tile([1, NT], F32, tag="zl", name=f"zl{nt}")
                    nc.scalar.activation(out=zl, in_=zf32, func=FT.Ln)
                    nc.scalar.activation(out=zrb, in_=zl, func=FT.Exp, scale=-1.0)
                else:
                    zr = zsp.tile([1, NT], F32, tag="zr", name=f"zr{nt}")
                    nc.vector.reciprocal(out=zr, in_=zf32)
                    nc.vector.tensor_copy(out=zrb, in_=zr)
                st["zbf"], st["zrb"] = zbf, zrb

            def tail2(nt, cp):
                """out-proj half cp: f = bo (x) Z + Wo^T @ osb; fraw; defer mul.
                On the last tile, half 1 borrows the freed zz bank so both
                halves' matmuls run without the serial pf rotation."""
                st = state[nt]
                csl = slice(cp * 128, (cp + 1) * 128)
                if nt == NTILES - 1 and cp == 1:
                    f_ps = pz.tile([128, NT], F32, tag="z", name=f"f{cp}_{nt}")
                else:
                    f_ps = pf.tile([128, NT], F32, tag="f", name=f"f{cp}_{nt}")
                nc.tensor.matmul(
                    f_ps, bo_sb[:, csl], st["zbf"], start=True, stop=False,
                )
                for cc in range(2):
                    nc.tensor.matmul(
                        f_ps, wo_sb[:, cc, csl], st[f"osb{cc}"],
                        start=False, stop=(cc == 1),
                    )
                fr = frp.tile([128, NT], BF16, tag=f"fr{cp}", name=f"fr{cp}_{nt}")
                nc.vector.tensor_copy(out=fr, in_=f_ps)
                st[f"fr{cp}"] = fr

            def tail2z(nt):
                """1/Z broadcast to 128 partitions (borrows the f slot);
                deferred past the reciprocal's latency. The final tile's TT
                reads the psum directly (nothing reuses the f slot after)."""
                st = state[nt]
                zb_ps = pf.tile([128, NT], F32, tag="f", name=f"zbp{nt}")
                nc.tensor.matmul(zb_ps, ones_m, st["zrb"], start=True, stop=True)
                if nt == NTILES - 1:
                    st["zb"] = zb_ps
                    return
                zb = zsp.tile([128, NT], BF16, tag="zb", name=f"zb{nt}")
                nc.vector.tensor_copy(out=zb, in_=zb_ps)
                st["zb"] = zb

            def tail3(nt, cp):
                st = state[nt]
                nsl = slice(nt * NT, (nt + 1) * NT)
                out_sb = outp.tile([128, NT], F32, tag="out", name=f"out{cp}_{nt}")
                nc.vector.tensor_tensor(
                    out=out_sb, in0=st[f"fr{cp}"], in1=st["zb"],
                    op=mybir.AluOpType.mult,
                )
                nc.sync.dma_start(out=outT[cp, :, nsl], in_=out_sb)

            def lag_target(nt, u):
                # after a tile boundary, hold back the new tile's first O'/Z
                # so ~3 T-packs of PE work cover the PSUM-evacuation WAR
                if nt == 0:
                    return 1
                return {1: 2, 2: 3, 3: 3, 4: 3, 5: 2}.get(u, 1)

            pending = []
            tq = {}  # (nt, u) -> emitted-ahead T psum tile

            def emit_T(nt, u):
                # T-packs are emitted one position ahead of their exp/OZ so
                # at tile boundaries the next tile's first T executes before
                # the previous tile's last O'/Z and ScalarE never drains
                g, s0 = u // 2, (2 * u) % 4
                t_ps = pt.tile([128, 2, NT], F32, tag="t", name=f"t{nt}_{u}")
                for j in range(2):
                    s = s0 + j
                    nc.tensor.matmul(
                        t_ps[:, j, :],
                        kt_stack[g][32 * s:32 * (s + 1), :],
                        qts[nt][32 * s:32 * (s + 1), :],
                        start=True, stop=True,
                        tile_position=(32 * s, 0),
                    )
                tq[(nt, u)] = t_ps

            def emit_unit(nt, u):
                t_ps = tq.pop((nt, u))
                e_sb = ep.tile([128, 2, NT], F8, tag="e", name=f"e{nt}_{u}")
                # exp split: ScalarE does cols [0, NT-WD), DVE does the last
                # WD via the fp8e4m3 bit-trick (u8 = round(s*EXPA + EXPB),
                # exact round-to-nearest; zero-mean so softmax stays untilted)
                nc.scalar.activation(
                    out=e_sb[:, :, 0:NT - WD], in_=t_ps[:, :, 0:NT - WD],
                    func=FT.Exp)
                if WD:
                    nc.vector.tensor_scalar(
                        out=e_sb[:, :, NT - WD:].bitcast(U8),
                        in0=t_ps[:, :, NT - WD:],
                        scalar1=EXPA, scalar2=EXPB,
                        op0=mybir.AluOpType.mult, op1=mybir.AluOpType.add)
                pending.append((nt, u, e_sb))
                # deferred tails for the PREVIOUS tile, emitted BEFORE the
                # O'/Z drain so their always-ready MMs sit ahead of work that
                # may still be waiting on exp
                if u == 13 and nt + 2 <= NTILES - 1:
                    qproj(nt + 2, f_psum)
                if nt > 0:
                    if u == 5:
                        tail2(nt - 1, 0)
                    elif u == 6:
                        tail2(nt - 1, 1)
                    elif u == 8:
                        tail2z(nt - 1)
                    elif u == 10:
                        tail3(nt - 1, 0)
                        tail3(nt - 1, 1)
                while len(pending) > lag_target(nt, u):
                    pnt, pu, pe = pending.pop(0)
                    emit_oz(pnt, pu, pe)
                    if pu == NU - 1:
                        tail1(pnt)      # right after the O'/Z stop

            # tile 0's units interleave with the tail of the projection chase
            # (K group g / V pairs land 2 rounds ahead of the units that use
            # them, so attention starts as soon as kstk0/qts0 are up)
            # once exp owns ScalarE, stop issuing DMA triggers from it
            dma_engs = [nc.sync, nc.gpsimd]

            sched = []
            for r in range(2, NGRP + 2):
                if r < NGRP:
                    sched.append(("proj", r))
                sched.append(("unit", (0, 2 * (r - 2))))
                sched.append(("unit", (0, 2 * (r - 2) + 1)))
            for nt in range(1, NTILES):
                for u in range(NU):
                    sched.append(("unit", (nt, u)))
            units = [a for k, a in sched if k == "unit"]
            emit_T(*units[0])
            ui = 0
            for kind, arg in sched:
                if kind == "proj":
                    r = arg
                    kproj(r)
                    # pairs 4-7 moved into the prologue bridge; rounds now
                    # cover pairs 8..15 (still >=2 rounds ahead of their use)
                    if 2 * r + 4 < NMB // 2:
                        vpair(2 * r + 4, lambda: proj_psum([128, C]))
                        vpair(2 * r + 5, lambda: proj_psum([128, C]))
                else:
                    ui += 1
                    if ui < len(units):
                        emit_T(*units[ui])
                    emit_unit(*arg)
            for pnt, pu, pe in pending:
                emit_oz(pnt, pu, pe)
                if pu == NU - 1:
                    tail1(pnt)
            for cp in range(2):
                tail2(NTILES - 1, cp)
            tail2z(NTILES - 1)
            for cp in range(2):
                tail3(NTILES - 1, cp)

    _split_multiwaits(nc)
    return nc


def _split_multiwaits(nc: bass.Bass) -> None:
    """This container's walrus accepts at most ONE sync-wait per instruction
    (CoreV3GenImpl setupSyncWait). Tile emits multi-wait instructions; split
    the excess waits onto EventSemaphore carriers inserted just before the
    instruction on the same engine."""
    import json as _json

    data = _json.loads(mybir.module_to_json_bytes(nc.m))
    uid = 0
    for fn in data["functions"]:
        for bb in fn["blocks"]:
            new = []
            for inst in bb["instructions"]:
                si = inst.get("sync_info")
                waits = (si or {}).get("on_wait") or []
                if len(waits) > 1:
                    for wcmd in waits[:-1]:
                        uid += 1
                        new.append({
                            "debug": inst.get("debug", 0),
                            "engine": inst["engine"],
                            "ins": [], "outs": [],
                            "name": f"syncw-{uid}",
                            "opcode": "EventSemaphore",
                            "sync_info": {"on_update": [], "on_wait": [wcmd]},
                        })
                    si["on_wait"] = [waits[-1]]
                new.append(inst)
            bb["instructions"] = new
    nc.m = mybir.module_from_json_bytes(_json.dumps(data).encode())


_NC = None


def _get_nc():
    global _NC
    if _NC is None:
        _NC = build_nc()
    return _NC


def _prep_maps(x, Wf, bf, Wg, bg, Wh, bh, Wo, bo):
    bft = ml_dtypes.bfloat16
    # DoubleRowSwInterleave reads weight columns reversed; store Wh/bh
    # channel-reversed within each 128-half so O' output channels land
    # straight (osb/Wo unchanged).
    rev = np.concatenate([np.arange(127, -1, -1), np.arange(255, 127, -1)])
    Whr = Wh[:, rev]
    bhr = bh[rev]
    shared = {
        "wf": np.ascontiguousarray(Wf.reshape(2, 128, KEY).astype(bft)),
        "wg": np.ascontiguousarray(Wg.reshape(2, 128, KEY).astype(bft)),
        "wh": np.ascontiguousarray(Whr.reshape(2, 128, C).astype(bft)),
        "wo": np.ascontiguousarray(Wo.reshape(2, 128, C).astype(bft)),
        "bfT": np.ascontiguousarray(bf.reshape(KEY, 1).astype(np.float32)),
        "bgT": np.ascontiguousarray(bg.reshape(KEY, 1).astype(np.float32)),
        "bhp": np.ascontiguousarray(bhr.reshape(1, C).astype(bft)),
        "bop": np.ascontiguousarray(bo.reshape(1, C).astype(bft)),
    }
    in_maps = []
    for b in range(B):
        xTb = np.ascontiguousarray(
            x[b].reshape(N, C).T.astype(bft).reshape(2, 128, N)
        )
        m = dict(shared)
        m["xT"] = xTb
        in_maps.append(m)
    return in_maps


def run(x, Wf, bf, Wg, bg, Wh, bh, Wo, bo, trace=False, **kw):
    x = np.asarray(x, dtype=np.float32)
    in_maps = _prep_maps(
        x, *(np.asarray(a, dtype=np.float32) for a in (Wf, bf, Wg, bg, Wh, bh, Wo, bo))
    )
    res = run_bass_kernel_spmd(_get_nc(), in_maps, list(range(B)), trace=trace, **kw)
    out = np.empty((B, H, W, C), dtype=np.float32)
    for b in range(B):
        oT = np.asarray(res.results[b]["outT"], dtype=np.float32).reshape(C, N)
        out[b] = oT.T.reshape(H, W, C)
    return out, res


def kernel(x, Wf, bf, Wg, bg, Wh, bh, Wo, bo):
    out, _ = run(x, Wf, bf, Wg, bg, Wh, bh, Wo, bo)
    return out



# revision 67
# speedup vs baseline: 1.0317x; 1.0317x over previous
"""SAGAN-style self-attention on 8 trn2 cores: data-parallel over batch.

Per core (one batch image): x^T [256,4096] bf16 in, out^T [256,4096] f32 out.
Projections (Q/K/V) chase the x DMAs (round-robin across SP/ACT/Pool
triggers); tile 0's attention units interleave with the tail of that chase.

  QT/KT = W^T @ xT + b        [32, 4096]  (bias via DVE tensor_scalar; K goes
                              straight into per-group kt_stack row strips)
  V     = x @ Wh + bh         fp8e4 pair tiles [128, 2, 256] ([keys, pair, c])
  per 512-query tile, per unit = pair of 128-key blocks (16 units/tile):
    T    = KT_strips.T @ QT    [128 keys, 2*512] 2-way row-packed (K=32),
                               t psum double-buffered so exp(u) || T(u+1)
    expT = exp(T)              ScalarE, PSUM->SBUF, fp8e4 (|s|<~5, no max-sub)
    O'  += V2_pair.T @ E2_pair [256, 512] fp8 DoubleRow PSUM accum
    Z   += ones2.T @ E2_pair   [16, 512] fp8 DoubleRow (rows identical)
  tail (deferred, off the PE critical path; O'/Z of the next tile are held
  back a few units so PSUM-evacuation WARs are covered by T-pack work):
    osb  = O' -> bf16 (DVE, frees o banks early); zf32 = Z -> SBUF (frees zz)
    zr   = 1/Z (DVE reciprocal, reads the SBUF copy)
    f    = bo (x) Z (K=1 preload) + Wo^T @ osb   (normalize AFTER projection:
    fraw = f -> bf16 (DVE, frees the f bank)      the division commutes)
    zb   = ones (x) (1/Z) broadcast [128, 512] (PE, borrows the f slot)
    out  = fraw * zb (DVE) -> DMA
PSUM banks: t=2x2 (double-buffered T), o0/o1=2, zz=1, f=1 -> 8 exactly.
fp8 E/V numerics: rel-L2 vs f64 reference = 1.47e-2 (gate 2e-2).

Tuning on top of the 215.9us baseline (measured 207.6us):
  - DMA triggers come off ScalarE once attention starts (exp owns ACT;
    each trigger costs ~600ns of the issuing engine's sequencer)
  - O' weights stored SW-interleaved ([keys, C, pair] with Wh/bh channel-
    reversed per 128-half on the host) and consumed with
    perf_mode=DoubleRowSwInterleave: weight loads read contiguously,
    LDWEIGHTS 213 -> ~90-130ns (O' issue rate itself is drain-bound, so
    the win is modest)
  - V pairs 4-7 are front-loaded into the prologue: their MMs bridge the
    PE-idle window while the kt_stack/qts strip DMAs land; without them
    the HAM MID window fires and the first ~10us of attention runs at
    K=4/8 (half clock)
  - rampdown tail (was 11.7us after the last O'): the final tile computes
    1/Z on ScalarE as exp(-ln Z) (Ln shares the exp table set, no
    ACT_TABLE_LOAD; the 3.3us single-lane DVE reciprocal was blocking the
    fr CASTs on the DVE queue), its out-multiply reads the zb broadcast
    psum directly, and its second out-proj half borrows the freed zz bank
    so both f halves' matmuls run without the serial pf-bank rotation
  - Z emitted BEFORE the unit's two O' matmuls (was after): Z's 32-col
    ones LDWEIGHTS pulls ahead during the T-pair streams and the Z matmul
    starts inside the T-pair's drain shadow, while the larger DRSI O'
    weight loads stream during Z's execution — hides most of Z's ~300ns
    (-8us). Emitting the next T-pack after the OZ group regresses 13us
    (ScalarE starves at boundaries) and emitting the deferred tails before
    the OZ drain regresses 6.5us, so both keep their original positions.
Measured 193.8-196.7us over four runs, rel-L2 identical to baseline.
"""

import sys

if "/opt/trn_rl_repo" not in sys.path:
    sys.path.insert(0, "/opt/trn_rl_repo")

import ml_dtypes
import numpy as np

import concourse.bass as bass
import concourse.mybir as mybir
import concourse.tile as tile
from concourse.bass_utils import run_bass_kernel_spmd

B, H, W, C = 8, 64, 64, 256
KEY = 32
N = H * W          # 4096 tokens
NT = 512           # query tile (free dim per matmul)
NTILES = N // NT   # 8
MB = 128           # key block
NMB = N // MB      # 32
GRP = 4            # key blocks per group (one per PE row strip)
NGRP = NMB // GRP  # 8

BF16 = mybir.dt.bfloat16
F32 = mybir.dt.float32
F8 = mybir.dt.float8e4
U8 = mybir.dt.uint8
FT = mybir.ActivationFunctionType
DR = mybir.MatmulPerfMode.DoubleRow
DRSI = mybir.MatmulPerfMode.DoubleRowSwInterleave

WD = 0             # exp fully on ScalarE (DVE insertion into the unit path regresses)
EXPA = 8.0 / float(np.log(2.0))
EXPB = 55.55       # zero-mean fp8e4m3 bit-trick bias (probe: exact u8 RN)


def build_nc() -> bass.Bass:
    nc = bass.Bass()

    xT = nc.declare_dram_parameter("xT", [2, 128, N], BF16, isOutput=False)
    wf = nc.declare_dram_parameter("wf", [2, 128, KEY], BF16, isOutput=False)
    wg = nc.declare_dram_parameter("wg", [2, 128, KEY], BF16, isOutput=False)
    wh = nc.declare_dram_parameter("wh", [2, 128, C], BF16, isOutput=False)
    wo = nc.declare_dram_parameter("wo", [2, 128, C], BF16, isOutput=False)
    bfT = nc.declare_dram_parameter("bfT", [KEY, 1], F32, isOutput=False)
    bgT = nc.declare_dram_parameter("bgT", [KEY, 1], F32, isOutput=False)
    bhp = nc.declare_dram_parameter("bhp", [1, C], BF16, isOutput=False)
    bop = nc.declare_dram_parameter("bop", [1, C], BF16, isOutput=False)
    outT = nc.declare_dram_parameter("outT", [2, 128, N], F32, isOutput=True)

    with tile.TileContext(nc) as tc:
        with (
            tc.tile_pool(name="const", bufs=1) as const,
            tc.tile_pool(name="xp", bufs=1) as xp,
            tc.tile_pool(name="vp", bufs=1) as vp,
            tc.tile_pool(name="qk", bufs=1) as qk,
            tc.tile_pool(name="ep", bufs=5) as ep,
            tc.tile_pool(name="osb", bufs=2) as osbp,
            tc.tile_pool(name="frp", bufs=2) as frp,
            tc.tile_pool(name="zsp", bufs=2) as zsp,
            tc.tile_pool(name="outp", bufs=3) as outp,
            tc.tile_pool(name="pt", bufs=2, space="PSUM") as pt,
            tc.tile_pool(name="po", bufs=1, space="PSUM") as po,
            tc.tile_pool(name="pz", bufs=1, space="PSUM") as pz,
            tc.tile_pool(name="pf", bufs=1, space="PSUM") as pf,
        ):
            # ---- constants ----
            ones2 = const.tile([128, 2, 16], F8)    # Z DoubleRow lhsT
            # (16-wide: DR weights need pair-step %16B == 0; rows identical)
            ones_m = const.tile([1, 128], BF16)     # K=1 broadcast lhsT
            nc.vector.memset(ones2, 1.0)
            nc.vector.memset(ones_m, 1.0)

            wf_sb = const.tile([128, 2, KEY], BF16)
            wg_sb = const.tile([128, 2, KEY], BF16)
            wh_sb = const.tile([128, 2, C], BF16)
            wo_sb = const.tile([128, 2, C], BF16)
            bf_sb = const.tile([KEY, 1], F32)
            bg_sb = const.tile([KEY, 1], F32)
            bh_sb = const.tile([1, C], BF16)
            bo_sb = const.tile([1, C], BF16)
            # DMA trigger round-robin across the three HWDGE/SWDGE engines:
            # each trigger costs ~600ns of that engine's sequencer, and a
            # single queue would serialize the whole input load (~40us).
            dma_i = 0
            dma_engs = [nc.sync, nc.scalar, nc.gpsimd]

            def dma_rr(out, in_):
                nonlocal dma_i
                eng = dma_engs[dma_i % len(dma_engs)]
                dma_i += 1
                eng.dma_start(out=out, in_=in_)
            # bh broadcast to all 128 partitions (for V bias add on DVE),
            # via K=1 ones matmul into a borrowed PSUM slot
            bh_bc = const.tile([128, C], BF16)

            # xT chunks in 512-col tiles (projections start on first slice)
            xts = [
                [xp.tile([128, NT], BF16, name=f"xt{cc}_{h}") for h in range(NTILES)]
                for cc in range(2)
            ]
            # x chunks first (K proj group g needs x tile h=g), weights
            # interleaved right behind the first x pair
            def xdma(h):
                for cc in range(2):
                    dma_rr(xts[cc][h], xT[cc, :, h * NT:(h + 1) * NT])

            xdma(0)
            for cc in range(2):
                dma_rr(wg_sb[:, cc, :], wg[cc])
                dma_rr(wf_sb[:, cc, :], wf[cc])
            dma_rr(bg_sb, bgT[:])
            dma_rr(bf_sb, bfT[:])
            xdma(1)
            for cc in range(2):
                dma_rr(wh_sb[:, cc, :], wh[cc])
            dma_rr(bh_sb, bhp[:])
            xdma(2)
            for cc in range(2):
                dma_rr(wo_sb[:, cc, :], wo[cc])
            dma_rr(bo_sb, bop[:])
            for h in range(3, NTILES):
                xdma(h)

            def xs(cc, start, width):
                h = start // NT
                assert (start + width - 1) // NT == h
                return xts[cc][h][:, start - h * NT: start - h * NT + width]

            pp_i = 0

            def proj_psum(shape):
                # borrow the "t"/"f" slots (alternating) before attention
                nonlocal pp_i
                pp_i += 1
                if pp_i % 2:
                    return pt.tile(shape, F32, tag="t", name=f"projps{pp_i}")
                return pf.tile(shape, F32, tag="f", name=f"projps{pp_i}")

            def emit_bh_bc():
                bh_ps = proj_psum([128, C])
                nc.tensor.matmul(bh_ps, ones_m, bh_sb, start=True, stop=True)
                nc.vector.tensor_copy(out=bh_bc, in_=bh_ps)

            # ---- K projection; per-group kt_stack tiles (so T of group g
            # only waits for group g), regrouped into row strips by
            # partition-shifting SBUF->SBUF DMAs
            kt_stack = [
                qk.tile([128, MB], BF16, name=f"kstk{g}") for g in range(NGRP)
            ]

            def kproj(g):
                ps = proj_psum([KEY, NT])
                for cc in range(2):
                    nc.tensor.matmul(
                        ps, wg_sb[:, cc, :], xs(cc, g * NT, NT),
                        start=(cc == 0), stop=(cc == 1),
                    )
                ktg = qk.tile([KEY, NT], BF16, name=f"kts{g}")
                nc.vector.tensor_scalar_add(ktg, ps, bg_sb)
                for i in range(4):
                    dma_rr(
                        kt_stack[g][32 * i:32 * (i + 1), :],
                        ktg[:, i * MB:(i + 1) * MB],
                    )

            # ---- per-query-tile Q tiles (replicated into 4 row strips) -----
            qts = [
                qk.tile([128, NT], BF16, name=f"qts{nt}") for nt in range(NTILES)
            ]

            def f_psum(shape):
                nonlocal pp_i
                pp_i += 1
                return pf.tile(shape, F32, tag="f", name=f"fps{pp_i}")

            def qproj(nt, psum_fn=None):
                ps = (psum_fn or proj_psum)([KEY, NT])
                for cc in range(2):
                    nc.tensor.matmul(
                        ps, wf_sb[:, cc, :], xs(cc, nt * NT, NT),
                        start=(cc == 0), stop=(cc == 1),
                    )
                nc.vector.tensor_scalar_add(qts[nt][0:KEY, :], ps, bf_sb)
                for i in range(1, 4):
                    nc.sync.dma_start(
                        out=qts[nt][32 * i:32 * (i + 1), :], in_=qts[nt][0:KEY, :]
                    )

            # ---- V projection -> fp8 pair tiles [128, 2, C] ----------------
            # V pair tiles in SW-interleaved layout [keys, C, pair]: the two
            # key blocks' values for a channel sit in adjacent bytes, so the
            # O' weight load reads contiguously (DoubleRowSwInterleave) and
            # skips DoubleRow's slow 2x interleaved LDWEIGHTS. The mode reads
            # weight columns reversed; host prep stores Wh/bh channel-reversed
            # within each 128-half so O' output partitions come out straight.
            v2 = [
                vp.tile([128, C, 2], F8, tag=f"v{p}", name=f"v{p}")
                for p in range(NMB // 2)
            ]

            def v2_lhsT(u, cc):
                # contiguous [128, 2, 128] descriptor over the interleaved
                # 256B half (pair stride 128B, col stride 1B)
                return (
                    v2[u][:, cc * 128:(cc + 1) * 128, :]
                    .rearrange("p m j -> p (m j)")
                    .rearrange("p (a b) -> p a b", a=2)
                )

            def vpair(pair, psum_fn):
                for mem in range(2):
                    mb = 2 * pair + mem
                    ps = psum_fn()
                    for cc in range(2):
                        nc.tensor.matmul(
                            ps, xs(cc, mb * MB, MB), wh_sb[:, cc, :],
                            start=(cc == 0), stop=(cc == 1),
                        )
                    nc.vector.tensor_tensor(
                        out=v2[pair][:, :, mem], in0=ps, in1=bh_bc,
                        op=mybir.AluOpType.add,
                    )

            # prologue: Q for tiles 0/1 and the first two K groups / four V
            # pairs; K groups 2..7 (+ V pairs) interleave with tile 0's units
            emit_bh_bc()
            qproj(0)
            qproj(1)
            for g in range(2):
                kproj(g)
                vpair(2 * g, lambda: proj_psum([128, C]))
                vpair(2 * g + 1, lambda: proj_psum([128, C]))
            # front-load V pairs 4-7: their x chunks are resident by now, so
            # these MMs bridge the PE-idle window while the kt_stack/qts strip
            # DMAs land. Without them PE idles ~2.3us waiting for the first T
            # pack, the HAM MID window fires, and the first ~10us of attention
            # runs at K=4/8 (half clock).
            for p4 in range(4, 8):
                vpair(p4, lambda: proj_psum([128, C]))

            # ---- attention: pipelined over (query-tile, pair-unit) ----------
            # unit u = one pair of key blocks (2u, 2u+1); 16 units per tile.
            # T psum is [128, 2*NT] (2 banks) double-buffered so exp(u) and
            # T(u+1) overlap; row strips alternate (0,1)/(2,3) across units.
            NU = NMB // 2  # 16
            state = {}  # nt -> dict with live tiles for the tail

            def emit_oz(nt, u, e_sb):
                if u == 0:
                    state[nt] = {
                        "o": [po.tile([128, NT], F32, tag="o0", name=f"o0_{nt}"),
                              po.tile([128, NT], F32, tag="o1", name=f"o1_{nt}")],
                        "zz": pz.tile([16, NT], F32, tag="z", name=f"z{nt}"),
                    }
                st = state[nt]
                first, last = u == 0, u == NU - 1
                nc.tensor.matmul(
                    st["zz"], ones2, e_sb,
                    start=first, stop=last,
                    perf_mode=DR,
                )
                for cc in range(2):
                    nc.tensor.matmul(
                        st["o"][cc],
                        v2_lhsT(u, cc),
                        e_sb,
                        start=first, stop=last,
                        perf_mode=DRSI,
                    )

            def tail1(nt):
                """PSUM evacuation; emit BEFORE next tile's first O'/Z.
                zz is freed by one fast copy (the slow reciprocal reads
                the SBUF copy later)."""
                st = state[nt]
                ot0 = osbp.tile([128, NT], BF16, tag="os0", name=f"os0_{nt}")
                nc.vector.tensor_copy(out=ot0, in_=st["o"][0])
                ot1 = osbp.tile([128, NT], BF16, tag="os1", name=f"os1_{nt}")
                nc.vector.tensor_copy(out=ot1, in_=st["o"][1])
                st["osb0"], st["osb1"] = ot0, ot1
                zf32 = zsp.tile([1, NT], F32, tag="zf32", name=f"zf32_{nt}")
                nc.vector.tensor_copy(out=zf32, in_=st["zz"][0:1, :])
                zbf = zsp.tile([1, NT], BF16, tag="zbf", name=f"zbf{nt}")
                nc.vector.tensor_copy(out=zbf, in_=zf32)
                zrb = zsp.tile([1, NT], BF16, tag="zrb", name=f"zrb{nt}")
                if nt == NTILES - 1:
                    # rampdown: the 3.3us DVE reciprocal would block the fr
                    # CASTs on the DVE queue; ScalarE is idle after the last
                    # exp, so compute 1/Z = exp(-ln Z) there instead
                    zl = zsp.tile([1, NT], F32, tag="zl", name=f"zl{nt}")
                    nc.scalar.activation(out=zl, in_=zf32, func=FT.Ln)
                    nc.scalar.activation(out=zrb, in_=zl, func=FT.Exp, scale=-1.0)
                else:
                    zr = zsp.tile([1, NT], F32, tag="zr", name=f"zr{nt}")
                    nc.vector.reciprocal(out=zr, in_=zf32)
                    nc.vector.tensor_copy(out=zrb, in_=zr)
                st["zbf"], st["zrb"] = zbf, zrb

            def tail2(nt, cp):
                """out-proj half cp: f = bo (x) Z + Wo^T @ osb; fraw; defer mul.
                On the last tile, half 1 borrows the freed zz bank so both
                halves' matmuls run without the serial pf rotation."""
                st = state[nt]
                csl = slice(cp * 128, (cp + 1) * 128)
                if nt == NTILES - 1 and cp == 1:
                    f_ps = pz.tile([128, NT], F32, tag="z", name=f"f{cp}_{nt}")
                else:
                    f_ps = pf.tile([128, NT], F32, tag="f", name=f"f{cp}_{nt}")
                nc.tensor.matmul(
                    f_ps, bo_sb[:, csl], st["zbf"], start=True, stop=False,
                )
                for cc in range(2):
                    nc.tensor.matmul(
                        f_ps, wo_sb[:, cc, csl], st[f"osb{cc}"],
                        start=False, stop=(cc == 1),
                    )
                fr = frp.tile([128, NT], BF16, tag=f"fr{cp}", name=f"fr{cp}_{nt}")
                nc.vector.tensor_copy(out=fr, in_=f_ps)
                st[f"fr{cp}"] = fr

            def tail2z(nt):
                """1/Z broadcast to 128 partitions (borrows the f slot);
                deferred past the reciprocal's latency. The final tile's TT
                reads the psum directly (nothing reuses the f slot after)."""
                st = state[nt]
                zb_ps = pf.tile([128, NT], F32, tag="f", name=f"zbp{nt}")
                nc.tensor.matmul(zb_ps, ones_m, st["zrb"], start=True, stop=True)
                if nt == NTILES - 1:
                    st["zb"] = zb_ps
                    return
                zb = zsp.tile([128, NT], BF16, tag="zb", name=f"zb{nt}")
                nc.vector.tensor_copy(out=zb, in_=zb_ps)
                st["zb"] = zb

            def tail3(nt, cp):
                st = state[nt]
                nsl = slice(nt * NT, (nt + 1) * NT)
                out_sb = outp.tile([128, NT], F32, tag="out", name=f"out{cp}_{nt}")
                nc.vector.tensor_tensor(
                    out=out_sb, in0=st[f"fr{cp}"], in1=st["zb"],
                    op=mybir.AluOpType.mult,
                )
                nc.sync.dma_start(out=outT[cp, :, nsl], in_=out_sb)

            def lag_target(nt, u):
                # after a tile boundary, hold back the new tile's first O'/Z
                # so ~3 T-packs of PE work cover the PSUM-evacuation WAR
                if nt == 0:
                    return 1
                return {1: 2, 2: 3, 3: 3, 4: 3, 5: 2}.get(u, 1)

            pending = []
            tq = {}  # (nt, u) -> emitted-ahead T psum tile

            def emit_T(nt, u):
                # T-packs are emitted one position ahead of their exp/OZ so
                # at tile boundaries the next tile's first T executes before
                # the previous tile's last O'/Z and ScalarE never drains
                g, s0 = u // 2, (2 * u) % 4
                t_ps = pt.tile([128, 2, NT], F32, tag="t", name=f"t{nt}_{u}")
                for j in range(2):
                    s = s0 + j
                    nc.tensor.matmul(
                        t_ps[:, j, :],
                        kt_stack[g][32 * s:32 * (s + 1), :],
                        qts[nt][32 * s:32 * (s + 1), :],
                        start=True, stop=True,
                        tile_position=(32 * s, 0),
                    )
                tq[(nt, u)] = t_ps

            def emit_unit(nt, u):
                t_ps = tq.pop((nt, u))
                e_sb = ep.tile([128, 2, NT], F8, tag="e", name=f"e{nt}_{u}")
                # exp split: ScalarE does cols [0, NT-WD), DVE does the last
                # WD via the fp8e4m3 bit-trick (u8 = round(s*EXPA + EXPB),
                # exact round-to-nearest; zero-mean so softmax stays untilted)
                nc.scalar.activation(
                    out=e_sb[:, :, 0:NT - WD], in_=t_ps[:, :, 0:NT - WD],
                    func=FT.Exp)
                if WD:
                    nc.vector.tensor_scalar(
                        out=e_sb[:, :, NT - WD:].bitcast(U8),
                        in0=t_ps[:, :, NT - WD:],
                        scalar1=EXPA, scalar2=EXPB,
                        op0=mybir.AluOpType.mult, op1=mybir.AluOpType.add)
                pending.append((nt, u, e_sb))
                while len(pending) > lag_target(nt, u):
                    pnt, pu, pe = pending.pop(0)
                    emit_oz(pnt, pu, pe)
                    if pu == NU - 1:
                        tail1(pnt)      # right after the O'/Z stop
                if u == 13 and nt + 2 <= NTILES - 1:
                    # Q for tile nt+2, off the critical path
                    qproj(nt + 2, f_psum)
                # deferred tails for the PREVIOUS tile
                if nt > 0:
                    if u == 5:
                        tail2(nt - 1, 0)
                    elif u == 6:
                        tail2(nt - 1, 1)
                    elif u == 8:
                        tail2z(nt - 1)
                    elif u == 10:
                        tail3(nt - 1, 0)
                        tail3(nt - 1, 1)

            # tile 0's units interleave with the tail of the projection chase
            # (K group g / V pairs land 2 rounds ahead of the units that use
            # them, so attention starts as soon as kstk0/qts0 are up)
            # once exp owns ScalarE, stop issuing DMA triggers from it
            dma_engs = [nc.sync, nc.gpsimd]

            sched = []
            for r in range(2, NGRP + 2):
                if r < NGRP:
                    sched.append(("proj", r))
                sched.append(("unit", (0, 2 * (r - 2))))
                sched.append(("unit", (0, 2 * (r - 2) + 1)))
            for nt in range(1, NTILES):
                for u in range(NU):
                    sched.append(("unit", (nt, u)))
            units = [a for k, a in sched if k == "unit"]
            emit_T(*units[0])
            ui = 0
            for kind, arg in sched:
                if kind == "proj":
                    r = arg
                    kproj(r)
                    # pairs 4-7 moved into the prologue bridge; rounds now
                    # cover pairs 8..15 (still >=2 rounds ahead of their use)
                    if 2 * r + 4 < NMB // 2:
                        vpair(2 * r + 4, lambda: proj_psum([128, C]))
                        vpair(2 * r + 5, lambda: proj_psum([128, C]))
                else:
                    ui += 1
                    if ui < len(units):
                        emit_T(*units[ui])
                    emit_unit(*arg)
            for pnt, pu, pe in pending:
                emit_oz(pnt, pu, pe)
                if pu == NU - 1:
                    tail1(pnt)
            for cp in range(2):
                tail2(NTILES - 1, cp)
            tail2z(NTILES - 1)
            for cp in range(2):
                tail3(NTILES - 1, cp)

    _split_multiwaits(nc)
    return nc


def _split_multiwaits(nc: bass.Bass) -> None:
    """This container's walrus accepts at most ONE sync-wait per instruction
    (CoreV3GenImpl setupSyncWait). Tile emits multi-wait instructions; split
    the excess waits onto EventSemaphore carriers inserted just before the
    instruction on the same engine."""
    import json as _json

    data = _json.loads(mybir.module_to_json_bytes(nc.m))
    uid = 0
    for fn in data["functions"]:
        for bb in fn["blocks"]:
            new = []
            for inst in bb["instructions"]:
                si = inst.get("sync_info")
                waits = (si or {}).get("on_wait") or []
                if len(waits) > 1:
                    for wcmd in waits[:-1]:
                        uid += 1
                        new.append({
                            "debug": inst.get("debug", 0),
                            "engine": inst["engine"],
                            "ins": [], "outs": [],
                            "name": f"syncw-{uid}",
                            "opcode": "EventSemaphore",
                            "sync_info": {"on_update": [], "on_wait": [wcmd]},
                        })
                    si["on_wait"] = [waits[-1]]
                new.append(inst)
            bb["instructions"] = new
    nc.m = mybir.module_from_json_bytes(_json.dumps(data).encode())


_NC = None


def _get_nc():
    global _NC
    if _NC is None:
        _NC = build_nc()
    return _NC


def _prep_maps(x, Wf, bf, Wg, bg, Wh, bh, Wo, bo):
    bft = ml_dtypes.bfloat16
    # DoubleRowSwInterleave reads weight columns reversed; store Wh/bh
    # channel-reversed within each 128-half so O' output channels land
    # straight (osb/Wo unchanged).
    rev = np.concatenate([np.arange(127, -1, -1), np.arange(255, 127, -1)])
    Whr = Wh[:, rev]
    bhr = bh[rev]
    shared = {
        "wf": np.ascontiguousarray(Wf.reshape(2, 128, KEY).astype(bft)),
        "wg": np.ascontiguousarray(Wg.reshape(2, 128, KEY).astype(bft)),
        "wh": np.ascontiguousarray(Whr.reshape(2, 128, C).astype(bft)),
        "wo": np.ascontiguousarray(Wo.reshape(2, 128, C).astype(bft)),
        "bfT": np.ascontiguousarray(bf.reshape(KEY, 1).astype(np.float32)),
        "bgT": np.ascontiguousarray(bg.reshape(KEY, 1).astype(np.float32)),
        "bhp": np.ascontiguousarray(bhr.reshape(1, C).astype(bft)),
        "bop": np.ascontiguousarray(bo.reshape(1, C).astype(bft)),
    }
    in_maps = []
    for b in range(B):
        xTb = np.ascontiguousarray(
            x[b].reshape(N, C).T.astype(bft).reshape(2, 128, N)
        )
        m = dict(shared)
        m["xT"] = xTb
        in_maps.append(m)
    return in_maps


def run(x, Wf, bf, Wg, bg, Wh, bh, Wo, bo, trace=False, **kw):
    x = np.asarray(x, dtype=np.float32)
    in_maps = _prep_maps(
        x, *(np.asarray(a, dtype=np.float32) for a in (Wf, bf, Wg, bg, Wh, bh, Wo, bo))
    )
    res = run_bass_kernel_spmd(_get_nc(), in_maps, list(range(B)), trace=trace, **kw)
    out = np.empty((B, H, W, C), dtype=np.float32)
    for b in range(B):
        oT = np.asarray(res.results[b]["outT"], dtype=np.float32).reshape(C, N)
        out[b] = oT.T.reshape(H, W, C)
    return out, res


def kernel(x, Wf, bf, Wg, bg, Wh, bh, Wo, bo):
    out, _ = run(x, Wf, bf, Wg, bg, Wh, bh, Wo, bo)
    return out

